# revision 77
# baseline (speedup 1.0000x reference)
"""Linear attention kernel for 8 Trainium2 NeuronCores.

Sharding: core = 2*b + hg  (b in 0..3 batches, hg in 0..1 head-groups of 8 heads).
Fully data-parallel - no collectives; host sums the two head-group partials per
batch and adds the bias.

Math per core (T=4096 tokens, CH=512 = 8 heads x 64, DIM=1024):
  pass 1 (per 512-token block):
    k = elu(x @ Wk)+1   token-major [T, CH]; q/k projections run in fp8
                        DoubleRow (2 contraction rows/cycle), v in bf16.
    v = x @ Wv          token-major
    kvT[j] += v^T k     per head-pair j, diagonal 64-blocks valid (PSUM held)
    zT[j]  += k_j^T @ 1 column of k sums, [128, 1] per pair  (PSUM held)
  transition:
    kvt <- diag blocks of kvT;  Zb[j] <- scatter of zT;  M[j] = kvt[j]^T @ W2
  pass 2 (software-pipelined per block: Q(i) | den(i-1) | y(i-2) | bc(i-1)):
    qT   = (x @ Wq)^T   c-major [CH, T], elu+1  (fp8 DoubleRow)
    den  = Zb^T qT      [8, T-block]
    r    = 1/(den+1e-6) (fast approx reciprocal)
    qsc  = qT * (E^T r) (broadcast r over each head's 64 rows)
    y    = qsc^T @ M    token-major [T, DIM], bf16 out

fp8 scales: x*8 and W*2048 before e4m3 cast; the elu eviction's activation
scale folds 2^-14 back out, so downstream tensors are true-scale.
"""

import sys

sys.path.insert(0, "/opt/trn_rl_repo")

import numpy as np

import concourse.bass as bass
import concourse.mybir as mybir
import concourse.tile as tile
from concourse import bacc

F32 = mybir.dt.float32
BF16 = mybir.dt.bfloat16
FP8 = mybir.dt.float8e4
AF = mybir.ActivationFunctionType
DR = mybir.MatmulPerfMode.DoubleRow

DIM = 1024      # model dim (contraction for projections)
CH = 512        # per-core channels (8 heads x 64)
P = 128
TB = 512        # tokens per block
NTB = 8

N_CORES = 8
B, T_FULL = 4, 4096

SX = 8.0        # fp8 pre-scale on x
SW = 2048.0     # fp8 pre-scale on W
SINV = 1.0 / (SX * SW)


def rr2(ap):
    """[128, 1024] fp8 slice -> [128, 2, 512] (contraction-pair major)."""
    return ap.rearrange("p (i f) -> p i f", i=2)


def build_nc(T=T_FULL):
    nc = bacc.Bacc(None, target_bir_lowering=False, debug=False)

    xbf = nc.declare_dram_parameter("xbf", [P, 8, T], BF16, isOutput=False)
    xdr = nc.declare_dram_parameter("xdr", [P, NTB, 8 * TB], FP8, isOutput=False)
    wk8 = nc.declare_dram_parameter("wk8", [P, 4096], FP8, isOutput=False)
    wq8 = nc.declare_dram_parameter("wq8", [P, 4096], FP8, isOutput=False)
    wv = nc.declare_dram_parameter("wv", [P, 4096], BF16, isOutput=False)
    w2 = nc.declare_dram_parameter("w2", [P, 4096], BF16, isOutput=False)
    ec = nc.declare_dram_parameter("ec", [8, CH], BF16, isOutput=False)
    y = nc.declare_dram_parameter("y", [T, DIM], BF16, isOutput=True)

    with tile.TileContext(nc) as tc:
        with (
            tc.tile_pool(name="persist", bufs=1) as pp,
            tc.tile_pool(name="p_xd", bufs=8) as xdp,
        ):
            ones_col = pp.tile([P, 1], BF16, name="ones_col", tag="ones_col")
            nc.vector.memset(ones_col[:, :], 1.0)

            wk8_sb = pp.tile([P, 4096], FP8, name="wk8_sb", tag="wk8_sb")
            wv_sb = pp.tile([P, 4096], BF16, name="wv_sb", tag="wv_sb")

            # loaded during pass 1 (after block-0/1 x tiles), used later
            wq8_sb = pp.tile([P, 4096], FP8, name="wq8_sb", tag="wq8_sb")
            w2_sb = pp.tile([P, 4096], BF16, name="w2_sb", tag="w2_sb")
            ec_sb = pp.tile([8, CH], BF16, name="ec_sb", tag="ec_sb")

            kvt = pp.tile([P, CH], BF16, name="kvt", tag="kvt")
            nc.vector.memset(kvt[:, :], 0.0)
            Zb = pp.tile([P, 32], BF16, name="Zb", tag="Zb")
            nc.vector.memset(Zb[:, :], 0.0)
            Ms = pp.tile([P, 4 * DIM], BF16, name="Ms", tag="Ms")

            xds = []
            with (
                tc.tile_pool(name="p1_x", bufs=3) as xp,
                tc.tile_pool(name="p1_sb", bufs=3) as pa,
                tc.tile_pool(name="p1_kv", bufs=2) as kvp,
                tc.tile_pool(name="p1_ps", bufs=4, space="PSUM") as pps,
                tc.tile_pool(name="hold_ps", bufs=1, space="PSUM") as hold_ps,
            ):
                kvps = hold_ps.tile([P, CH], F32, name="kvps", tag="kvps")
                ztp = hold_ps.tile([P, 4], F32, name="ztp", tag="ztp")
                nc.vector.memset(kvps[:, :], 0.0)
                nc.vector.memset(ztp[:, :], 0.0)

                k_sbs, v_sbs, xbs = {}, {}, {}

                def kvz(b, t):
                    last = b == NTB - 1
                    for j in range(4):
                        src = slice(t * CH + j * P, t * CH + (j + 1) * P)
                        nc.tensor.matmul(
                            kvps[:, j * P:(j + 1) * P], v_sbs[b][:, src], k_sbs[b][:, src],
                            start=False, stop=(last and t == 0 and j == 3),
                            skip_group_check=True,
                        )
                    for j in range(4):
                        src = slice(t * CH + j * P, t * CH + (j + 1) * P)
                        nc.tensor.matmul(
                            ztp[:, j:j + 1], k_sbs[b][:, src], ones_col[:, :],
                            start=False, stop=(last and t == 0),
                            skip_group_check=True,
                        )

                def vkvz(b):
                    # v t3-first + kv(t) after v(t-1)'s eviction so the PE
                    # never waits on an eviction that just stopped
                    for t in range(3, -1, -1):
                        csl = slice(t * CH, (t + 1) * CH)
                        vps = pps.tile([P, CH], F32, name=f"vps_{b}_{t}", tag="vps", bufs=2)
                        for ct in range(8):
                            nc.tensor.matmul(
                                vps[:, :],
                                xbs[b][:, ct * TB + t * P: ct * TB + (t + 1) * P],
                                wv_sb[:, ct * CH:(ct + 1) * CH],
                                start=(ct == 0), stop=(ct == 7),
                            )
                        nc.scalar.copy(v_sbs[b][:, csl], vps[:, :])
                        if t <= 2:
                            kvz(b, t + 1)
                    kvz(b, 0)

                for ib in range(NTB):
                    xd = xdp.tile([P, 8 * TB], FP8, name=f"xd_{ib}", tag="xd")
                    xds.append(xd)
                    xb = xp.tile([P, 8 * TB], BF16, name=f"xb_{ib}", tag="xb")
                    if ib == 0:
                        # fine-grained interleave so the first k matmul only
                        # waits on 256KB and the v accumulation streams in;
                        # kicks split across the two HWDGE engines (SP + ACT)
                        for g in range(4):
                            gsl = slice(g * 1024, (g + 1) * 1024)
                            nc.sync.dma_start(out=wk8_sb[:, gsl], in_=wk8[:, gsl])
                            nc.scalar.dma_start(out=xd[:, gsl], in_=xdr[:, ib, gsl])
                        for ct in range(8):
                            csl_ = slice(ct * TB, (ct + 1) * TB)
                            nc.sync.dma_start(
                                out=xb[:, csl_],
                                in_=xbf[:, ct:ct + 1, ib * TB:(ib + 1) * TB],
                            )
                            nc.scalar.dma_start(out=wv_sb[:, csl_], in_=wv[:, csl_])
                    else:
                        nc.sync.dma_start(out=xd[:, :], in_=xdr[:, ib, :])
                        nc.sync.dma_start(
                            out=xb[:, :].rearrange("p (c f) -> p c f", c=8),
                            in_=xbf[:, :, ib * TB:(ib + 1) * TB],
                        )
                    if ib == 1:
                        nc.sync.dma_start(out=w2_sb[:, :], in_=w2[:, :])
                        nc.sync.dma_start(out=wq8_sb[:, :], in_=wq8[:, :])
                        nc.sync.dma_start(out=ec_sb[:, :], in_=ec[:, :])

                    k_sb = kvp.tile([P, 4 * CH], BF16, name=f"k_{ib}", tag="k_sb")
                    v_sb = kvp.tile([P, 4 * CH], BF16, name=f"v_{ib}", tag="v_sb")
                    k_sbs[ib] = k_sb
                    v_sbs[ib] = v_sb
                    xbs[ib] = xb

                    # k projection, fp8 DoubleRow
                    for t in range(4):
                        tsl = slice(t * P, (t + 1) * P)
                        csl = slice(t * CH, (t + 1) * CH)
                        kps = pps.tile([P, CH], F32, name=f"kps_{ib}_{t}", tag="kps", bufs=2)
                        for g in range(4):
                            gsl = slice(g * 1024, (g + 1) * 1024)
                            nc.tensor.matmul(
                                kps[:, :],
                                rr2(xd[:, gsl])[:, :, tsl],
                                rr2(wk8_sb[:, gsl]),
                                start=(g == 0), stop=(g == 3),
                                perf_mode=DR,
                            )
                        km = pa.tile([P, CH], BF16, name=f"km_{ib}_{t}", tag="km")
                        ke = pa.tile([P, CH], BF16, name=f"ke_{ib}_{t}", tag="ke")
                        kr = pa.tile([P, CH], BF16, name=f"kr_{ib}_{t}", tag="kr")
                        nc.vector.tensor_scalar_min(km[:, :], kps[:, :], 0.0)
                        nc.scalar.activation(ke[:, :], km[:, :], AF.Exp, scale=SINV)
                        nc.scalar.activation(kr[:, :], kps[:, :], AF.Relu, scale=SINV)
                        nc.gpsimd.tensor_add(k_sb[:, csl], ke[:, :], kr[:, :])

                    # v projection + kv/zT run one block BEHIND k so the
                    # early-ramp v stalls (xb/wv DMA) hide under k work
                    if ib >= 1:
                        vkvz(ib - 1)
                vkvz(NTB - 1)

                # ---- transition: evict kv diag blocks, Zb scatter ----
                for j in range(4):
                    eng = nc.vector if j < 2 else nc.scalar
                    cp = eng.tensor_copy if j < 2 else eng.copy
                    cp(
                        kvt[0:64, j * P:j * P + 64], kvps[0:64, j * P:j * P + 64]
                    )
                    cp(
                        kvt[64:128, j * P + 64:(j + 1) * P],
                        kvps[64:128, j * P + 64:(j + 1) * P],
                    )
                    nc.scalar.copy(
                        Zb[0:64, j * 8 + 2 * j:j * 8 + 2 * j + 1], ztp[0:64, j:j + 1]
                    )
                    nc.scalar.copy(
                        Zb[64:128, j * 8 + 2 * j + 1:j * 8 + 2 * j + 2],
                        ztp[64:128, j:j + 1],
                    )

            with (
                tc.tile_pool(name="p2_sb", bufs=3) as pb,
                tc.tile_pool(name="p2_qt", bufs=3) as qtp,
                tc.tile_pool(name="p2_qsc", bufs=2) as qscp,
                tc.tile_pool(name="p2_y", bufs=3) as yp_sb,
                tc.tile_pool(name="p2_ps", bufs=2, space="PSUM") as bps,
            ):
                qts, qscs = {}, {}
                for it in range(NTB + 2):
                    # M[j] = kvt[j]^T @ W2[j], deferred past Q(0) so the kvt
                    # eviction copies have a Q-projection's time to land
                    if it == 1:
                        for j in range(4):
                            for h in range(2):
                                mps = bps.tile([P, CH], F32, name=f"mps_{j}_{h}", tag="yps", bufs=4)
                                nc.tensor.matmul(
                                    mps[:, :], kvt[:, j * P:(j + 1) * P],
                                    w2_sb[:, j * 1024 + h * CH: j * 1024 + (h + 1) * CH],
                                    start=True, stop=True,
                                )
                                if h == 0:
                                    nc.vector.tensor_copy(
                                        Ms[:, j * DIM + h * CH: j * DIM + (h + 1) * CH], mps[:, :]
                                    )
                                else:
                                    nc.scalar.copy(
                                        Ms[:, j * DIM + h * CH: j * DIM + (h + 1) * CH], mps[:, :]
                                    )
                    # ---- Q(it): fp8 DoubleRow projection, c-major ----
                    if it < NTB:
                        xd2 = xds[it]
                        qt = qtp.tile([P, 4 * CH], BF16, name=f"qt_{it}", tag="qt")
                        qts[it] = qt
                        for j in range(4):
                            jsl = slice(j * P, (j + 1) * P)
                            qps = bps.tile([P, CH], F32, name=f"qps_{it}_{j}", tag="qps", bufs=2)
                            for g in range(4):
                                gsl = slice(g * 1024, (g + 1) * 1024)
                                nc.tensor.matmul(
                                    qps[:, :],
                                    rr2(wq8_sb[:, gsl])[:, :, jsl],
                                    rr2(xd2[:, gsl]),
                                    start=(g == 0), stop=(g == 3),
                                    perf_mode=DR,
                                )
                            qm = pb.tile([P, CH], BF16, name=f"qm_{it}_{j}", tag="qm")
                            qe = pb.tile([P, CH], BF16, name=f"qe_{it}_{j}", tag="qe")
                            qr = pb.tile([P, CH], BF16, name=f"qr_{it}_{j}", tag="qr")
                            nc.vector.tensor_scalar_min(qm[:, :], qps[:, :], 0.0)
                            nc.scalar.activation(qe[:, :], qm[:, :], AF.Exp, scale=SINV)
                            nc.scalar.activation(qr[:, :], qps[:, :], AF.Relu, scale=SINV)
                            nc.gpsimd.tensor_add(qt[:, j * CH:(j + 1) * CH], qe[:, :], qr[:, :])

                    # ---- den(it-1) + reciprocal (early in the DVE queue so
                    # the bc matmuls issued after y(it-2) never wait) ----
                    if 1 <= it <= NTB:
                        pbk = it - 1
                        qt1 = qts[pbk]
                        dpt = bps.tile([P, CH], F32, name=f"dps_{pbk}", tag="yps", bufs=4)
                        dps = dpt[0:8, :]
                        for j in range(4):
                            nc.tensor.matmul(
                                dps, Zb[:, j * 8:(j + 1) * 8],
                                qt1[:, j * CH:(j + 1) * CH],
                                start=(j == 0), stop=(j == 3),
                            )
                        rr = pb.tile([8, CH], F32, name=f"rr_{pbk}", tag="rr")
                        nc.vector.reciprocal_approx_fast(out=rr[:, :], in_=dps)
                        rT = pb.tile([8, CH], BF16, name=f"rT_{pbk}", tag="rT")
                        nc.vector.tensor_copy(rT[:, :], rr[:, :])

                    # ---- y(it-2) ----
                    if it >= 2:
                        ybk = it - 2
                        qsc = qscs.pop(ybk)
                        for t in range(4):
                            row = (ybk * 4 + t) * P
                            y_sb = yp_sb.tile([P, DIM], BF16, name=f"y_{ybk}_{t}", tag="y_sb")
                            for h in range(2):
                                hsl = slice(h * CH, (h + 1) * CH)
                                yp = bps.tile([P, CH], F32, name=f"yps_{ybk}_{t}_{h}", tag="yps", bufs=4)
                                for j in range(4):
                                    nc.tensor.matmul(
                                        yp[:, :],
                                        qsc[:, j * CH + t * P: j * CH + (t + 1) * P],
                                        Ms[:, j * DIM + h * CH: j * DIM + (h + 1) * CH],
                                        start=(j == 0), stop=(j == 3),
                                    )
                                if h == 0:
                                    nc.vector.tensor_copy(y_sb[:, hsl], yp[:, :])
                                else:
                                    nc.scalar.copy(y_sb[:, hsl], yp[:, :])
                            nc.sync.dma_start(out=y[row:row + P, :], in_=y_sb[:, :])

                    # ---- bc(it-1) + qsc.  den ~1e5 > 0 so the reference's
                    # +1e-6 is numerically irrelevant. ----
                    if 1 <= it <= NTB:
                        pbk = it - 1
                        qt1 = qts.pop(pbk)
                        qsc = qscp.tile([P, 4 * CH], BF16, name=f"qsc_{pbk}", tag="qsc")
                        qscs[pbk] = qsc
                        for j in range(4):
                            bcp = bps.tile([P, CH], F32, name=f"bcp_{pbk}_{j}", tag="bcp", bufs=2)
                            nc.tensor.matmul(
                                bcp[:, :], ec_sb[:, j * P:(j + 1) * P], rT[:, :],
                                start=True, stop=True,
                            )
                            nc.vector.tensor_mul(
                                qsc[:, j * CH:(j + 1) * CH],
                                qt1[:, j * CH:(j + 1) * CH], bcp[:, :],
                            )

    nc.compile()
    return nc


_NC_CACHE = {}


def _get_nc(T=T_FULL):
    if T not in _NC_CACHE:
        _NC_CACHE[T] = build_nc(T)
    return _NC_CACHE[T]


def make_in_maps(x, W_qkv, W_out, b_out):
    import ml_dtypes

    bf16 = ml_dtypes.bfloat16
    e4 = ml_dtypes.float8_e4m3
    x = np.asarray(x, dtype=np.float32)
    W_qkv = np.asarray(W_qkv, dtype=np.float32)
    W_out = np.asarray(W_out, dtype=np.float32)

    xbfs, xdrs = [], []
    for b in range(B):
        xs = x[b]  # [T, DIM]
        xbfs.append(
            np.ascontiguousarray(
                xs.T.astype(bf16).reshape(8, P, T_FULL).transpose(1, 0, 2)
            )
        )
        t8 = (xs * SX).astype(e4)  # [T, DIM]
        xdrs.append(
            np.ascontiguousarray(
                t8.reshape(NTB, TB, 4, 2, P)
                .transpose(4, 0, 2, 3, 1)
                .reshape(P, NTB, 8 * TB)
            )
        )

    def pack8(w):  # [DIM, CH] fp8-scaled -> [P, 4096]
        return np.ascontiguousarray(
            (w * SW).astype(e4).reshape(4, 2, P, CH).transpose(2, 0, 1, 3).reshape(P, 4096)
        )

    def packb(w):  # [DIM, CH] -> [P, 4096] bf16 c-tile major
        return np.ascontiguousarray(
            w.astype(bf16).reshape(8, P, CH).transpose(1, 0, 2).reshape(P, 4096)
        )

    wq8s, wk8s, wvs, w2s = [], [], [], []
    for hg in range(2):
        cs = slice(hg * CH, (hg + 1) * CH)
        wq8s.append(pack8(W_qkv[:, hg * CH:(hg + 1) * CH]))
        wk8s.append(pack8(W_qkv[:, DIM + hg * CH: DIM + (hg + 1) * CH]))
        wvs.append(packb(W_qkv[:, 2 * DIM + hg * CH: 2 * DIM + (hg + 1) * CH]))
        w2s.append(
            np.ascontiguousarray(
                W_out[cs, :].astype(bf16).reshape(4, P, DIM).transpose(1, 0, 2).reshape(P, 4096)
            )
        )
    ecm = make_ec().astype(bf16)

    in_maps = []
    for core in range(N_CORES):
        b, hg = core // 2, core % 2
        in_maps.append({
            "xbf": xbfs[b], "xdr": xdrs[b],
            "wk8": wk8s[hg], "wq8": wq8s[hg], "wv": wvs[hg],
            "w2": w2s[hg], "ec": ecm,
        })
    return in_maps


def make_ec():
    """E selector: ec[h, j*128+p] = 1 iff head-of-partition-p-in-tile-j == h."""
    ecm = np.zeros((8, CH), dtype=np.float32)
    for j in range(4):
        ecm[2 * j, j * P:j * P + 64] = 1.0
        ecm[2 * j + 1, j * P + 64:(j + 1) * P] = 1.0
    return ecm


def kernel(x, W_qkv, W_out, b_out):
    from concourse.bass_utils import run_bass_kernel_spmd

    nc = _get_nc(T_FULL)
    in_maps = make_in_maps(x, W_qkv, W_out, b_out)
    res = run_bass_kernel_spmd(nc, in_maps, core_ids=list(range(N_CORES))).results
    bo = np.asarray(b_out, dtype=np.float32)
    out = np.empty((B, T_FULL, DIM), dtype=np.float32)
    for b in range(B):
        out[b] = (
            res[2 * b]["y"].astype(np.float32)
            + res[2 * b + 1]["y"].astype(np.float32)
            + bo
        )
    return out
